# revision 13
# baseline (speedup 1.0000x reference)
"""GAT layer kernel for Trainium2 (8 NeuronCores, SPMD).

Math note: in the reference,
    att = softmax(scores, axis=1); w = att.sum(axis=1)
sums the softmax over the exact axis it normalizes, so w == 1 identically
(up to fp rounding).  The layer therefore reduces to
    out[v] = (1/H) * ( (sum_{e: dst[e]==v} x[src[e]]) @ W  +  deg_in(v) * b )
i.e. a sparse neighbor aggregation (gather + segment-sum) followed by a
small dense matmul.  This is memory-bound in the per-edge row gather,
which is what the kernel optimizes.

Sharding: edges are partitioned by dst-node range (6250 nodes per core), so
each core owns the full accumulation for its node slice and the final
output is a pure concatenation -- no inter-core collective needed.

Device pipeline per core:
  - edges are grouped by 128-node output tile ("group") and src half
    (gather indices are int16, so x is split in two row halves); each
    (group, half) segment is padded to a multiple of 128 edge slots (pad
    slots gather row 0 and carry dst offset -1 so they contribute exactly
    nothing to the MACs).
  - the x table is fp16: halves HBM bytes; accumulation stays fp32, so
    the only loss is the one-off fp16 quantization of x (~2e-4 relative).
  - segments of BUNDLE adjacent groups share one dma_gather per half
    (~2K rows per call, the SWDGE sweet spot) spread over 4 SWDGE queues.
    The Q7 descriptor-prep rate (~2.6-3.3 ns/row on the two full-reach
    Q7 cores) is the hard floor of this kernel.
  - segment-sum on the tensor engine, one matmul per 128-edge subtile:
      ypsum_g[f, v] += xg[e, f]^T @ onehot[e, v]
    with the one-hot from a vector-engine is_equal of an iota row against
    the per-edge group-local dst offset.
  - per group, PSUM y^T is multiplied by W/4 (K=128 matmul) plus a K=1
    outer-product matmul for the deg*b/4 term, and streamed out.
"""

import numpy as np

import concourse.bass as bass
import concourse.tile as tile
from concourse import bacc, mybir
from concourse.bass_utils import run_bass_kernel_spmd

F32 = mybir.dt.float32
I16 = mybir.dt.int16
GDT = mybir.dt.float16   # gather-table / one-hot dtype
GNP = np.float16

N_NODES = 50000
N_EDGES = 800000
D = 128          # in feats == H*F
HEADS = 4
N_CORES = 8
NPC = N_NODES // N_CORES      # nodes per core
P = 128                       # partitions / node-group size
BUNDLE = 2                    # groups whose segments share one gather


def _prep(x, weight, bias, src, dst, n_cores, npc):
    """Host-side sharding; returns per-core input maps + static tiling."""
    n_nodes, d = x.shape
    ng = (npc + P - 1) // P
    nb = (ng + BUNDLE - 1) // BUNDLE
    half = (n_nodes + 1) // 2
    assert half < 32768 and (n_nodes - half) < 32768

    src64 = src.astype(np.int64)
    dst64 = dst.astype(np.int64)
    core = dst64 // npc
    loc_node = dst64 % npc
    g_e = loc_node // P
    loc = (loc_node % P).astype(np.float32)
    h_e = (src64 >= half).astype(np.int64)

    key = (core * ng + g_e) * 2 + h_e
    order = np.argsort(key, kind="stable")
    key_s = key[order]
    src_s = src64[order]
    loc_s = loc[order]

    cnt = np.bincount(key, minlength=n_cores * ng * 2).reshape(n_cores, ng, 2)
    nk2 = (-(-cnt.max(axis=0) // P)).astype(np.int64)  # [ng, 2] tiles/segment

    # subtile order: per bundle of BUNDLE groups, lo segments then hi.
    # Within each (bundle, half) the group with the LARGEST tile envelope
    # goes last: its per-core padding is then a trailing run in the gather's
    # index stream, marked -1 so the Q7 prep (and DMA) skips it entirely.
    seg_t0 = np.zeros((ng, 2), np.int64)
    chunks = []  # (bundle, half, t0, ntiles, last_group)
    t = 0
    for b in range(nb):
        gb = list(range(b * BUNDLE, min(ng, (b + 1) * BUNDLE)))
        for hh in (0, 1):
            t0c = t
            gbs = sorted(gb, key=lambda g: int(nk2[g, hh]))
            for gg in gbs:
                seg_t0[gg, hh] = t
                t += int(nk2[gg, hh])
            if t > t0c:
                chunks.append((b, hh, t0c, t - t0c, gbs[-1]))
    T = t

    seg_start = np.zeros(n_cores * ng * 2, np.int64)
    seg_start[1:] = np.cumsum(cnt.reshape(-1))[:-1]
    rank = np.arange(len(src_s), dtype=np.int64) - seg_start[key_s]
    c_e = key_s // (ng * 2)
    gg_e = (key_s // 2) % ng
    t_e = seg_t0[gg_e, key_s % 2] + rank // P
    p_e = rank % P

    dstoff = np.full((n_cores, P, T), -1.0, GNP)
    dstoff[c_e, p_e, t_e] = loc_s.astype(GNP)

    # int16 indices: slot (p, t) -> idx16[p % 16, 8*t + p//16]
    idx16 = np.zeros((n_cores, 16, 8 * T), np.int16)
    hval = (src_s - (key_s % 2) * half).astype(np.int16)
    idx16[c_e, p_e % 16, 8 * t_e + p_e // 16] = hval
    # trailing pads of each chunk's LAST segment -> -1 (ucode trims trailing
    # negative idxs: no descriptors are generated or transferred for them).
    # Interior pads keep idx 0 (gather row 0; their one-hot column is zero).
    # nvalid[c, chunk] = number of remaining (non-negative) idxs; the ucode
    # contract is num_idxs_reg == that count (ring reservation must match
    # generation), and it is per-core data -> runtime register.
    nvalid = np.zeros((n_cores, len(chunks)), np.int32)
    for ci, (_b, hh, t0c, nt, gl) in enumerate(chunks):
        for c in range(n_cores):
            start = int(cnt[c, gl, hh])
            end = int(nk2[gl, hh]) * P
            if start == 0 and end > 0:
                start = 1  # keep one valid idx so the gather is never empty
            nvalid[c, ci] = nt * P - max(0, end - start)
            if start >= end:
                continue
            r = np.arange(start, end)
            tt = seg_t0[gl, hh] + r // P
            pp = r % P
            idx16[c, pp % 16, 8 * tt + pp // 16] = -1
    idx16 = np.ascontiguousarray(np.tile(idx16, (1, 8, 1)))

    deg = np.bincount(dst64, minlength=n_nodes).astype(np.float32)
    deg4 = np.zeros((n_cores, 1, ng * P), np.float32)
    deg4[:, 0, :npc] = (deg / HEADS).reshape(n_cores, npc)

    iota = np.broadcast_to(np.arange(P, dtype=GNP)[None, :], (P, P)).copy()
    w4 = np.ascontiguousarray(weight.astype(np.float32) / HEADS)
    b4 = bias.astype(np.float32).reshape(1, d)  # deg4 already carries the /H
    xg16 = np.ascontiguousarray(x.astype(GNP))
    xlo = np.ascontiguousarray(xg16[:half])
    xhi = np.ascontiguousarray(xg16[half:])

    in_maps = []
    for c in range(n_cores):
        in_maps.append(
            {
                "xlo": xlo,
                "xhi": xhi,
                "idx": idx16[c],
                "dstoff": dstoff[c],
                "iota": iota,
                "w4": w4,
                "b4": b4,
                "deg4": deg4[c],
                "nv": np.ascontiguousarray(nvalid[c].reshape(1, -1)),
            }
        )
    meta = dict(nk2=nk2, seg_t0=seg_t0, chunks=chunks, T=T, ng=ng,
                nb=nb, half=half)
    return in_maps, meta


def _build(n_nodes, d, npc, meta):
    nk2, seg_t0, chunks, T, ng, nb, half = (
        meta["nk2"], meta["seg_t0"], meta["chunks"], meta["T"],
        meta["ng"], meta["nb"], meta["half"],
    )
    nc = bacc.Bacc("TRN2", num_swdge_queues=4)
    xlo_d = nc.dram_tensor("xlo", [half, d], GDT, kind="ExternalInput")
    xhi_d = nc.dram_tensor("xhi", [n_nodes - half, d], GDT, kind="ExternalInput")
    idx_d = nc.dram_tensor("idx", [P, 8 * T], I16, kind="ExternalInput")
    dstoff_d = nc.dram_tensor("dstoff", [P, T], GDT, kind="ExternalInput")
    iota_d = nc.dram_tensor("iota", [P, P], GDT, kind="ExternalInput")
    w4_d = nc.dram_tensor("w4", [d, d], F32, kind="ExternalInput")
    b4_d = nc.dram_tensor("b4", [1, d], F32, kind="ExternalInput")
    deg4_d = nc.dram_tensor("deg4", [1, ng * P], F32, kind="ExternalInput")
    nchunks = len(chunks)
    nv_d = nc.dram_tensor("nv", [1, nchunks], mybir.dt.int32, kind="ExternalInput")
    out_d = nc.dram_tensor("out", [npc, d], F32, kind="ExternalOutput")

    x_in = [xlo_d, xhi_d]
    cap = [1, 1]
    for (_b, hh, _t0, nt, _gl) in chunks:
        cap[hh] = max(cap[hh], nt)
    chunk_of = {}
    for ci, (b, hh, t0c, nt, _gl) in enumerate(chunks):
        chunk_of[(b, hh)] = (t0c, nt, ci)

    with tile.TileContext(nc) as tc:
        with (
            tc.tile_pool(name="consts", bufs=1) as cpool,
            tc.tile_pool(name="xglo", bufs=6) as gpool_lo,
            tc.tile_pool(name="xghi", bufs=6) as gpool_hi,
            tc.tile_pool(name="ind", bufs=32) as ipool,
            tc.tile_pool(name="ysb", bufs=4) as ypool,
            tc.tile_pool(name="osb", bufs=4) as opool,
            tc.tile_pool(name="ypsum", bufs=6, space="PSUM") as yppool,
            tc.tile_pool(name="opsum", bufs=2, space="PSUM") as oppool,
        ):
            # piecewise index tables as INDEPENDENT tiles (Tile tracks deps
            # per tile, so the first gather must not wait on the whole
            # table).  Piece boundaries align with chunk boundaries and are
            # GEOMETRIC: tiny first pieces so gather #0 starts ~2us in, big
            # later pieces once there's pipeline runway.
            target = 4
            bndl = [0]
            for (_b, _hh, t0c, nt, _gl) in chunks:
                if t0c + nt - bndl[-1] >= target and t0c + nt < T:
                    bndl.append(t0c + nt)
                    target = min(target * 2, 160)
            bndl.append(T)
            idx_p, dst_p = [], []
            for i in range(len(bndl) - 1):
                lo, hi = bndl[i], bndl[i + 1]
                ip = cpool.tile([P, 8 * (hi - lo)], I16, name=f"idxp{i}")
                dp = cpool.tile([P, hi - lo], GDT, name=f"dstp{i}")
                idx_p.append(ip)
                dst_p.append(dp)

            iota_sb = cpool.tile([P, P], GDT)
            w4_sb = cpool.tile([d, d], F32)
            b4_sb = cpool.tile([1, d], F32)
            deg4_sb = cpool.tile([1, ng * P], F32)

            # DMA issue order: first two pieces, then the small constants the
            # early compute needs, then the remaining (large) pieces.
            def _load_piece(i):
                lo, hi = bndl[i], bndl[i + 1]
                nc.sync.dma_start(out=idx_p[i][:], in_=idx_d[:, 8 * lo : 8 * hi])
                nc.sync.dma_start(out=dst_p[i][:], in_=dstoff_d[:, lo:hi])

            nv_sb = cpool.tile([1, nchunks], mybir.dt.int32, name="nv")
            nc.sync.dma_start(out=nv_sb[:], in_=nv_d[:])
            for i in range(min(2, len(idx_p))):
                _load_piece(i)
            nc.sync.dma_start(out=iota_sb[:], in_=iota_d[:])
            nc.sync.dma_start(out=w4_sb[:], in_=w4_d[:])
            nc.sync.dma_start(out=b4_sb[:], in_=b4_d[:])
            nc.sync.dma_start(out=deg4_sb[:], in_=deg4_d[:])
            for i in range(2, len(idx_p)):
                _load_piece(i)

            # per-chunk valid-idx counts -> Pool registers, in 2 banks of
            # RBANK loaded >= RBANK gathers ahead of use (the sequencer can
            # run ahead of the engine by at most ~9 instructions, so a bank
            # is never overwritten while a pending gather still reads it).
            RBANK = 16
            nv_regs = [
                nc.alloc_register(mybir.EngineType.Pool, f"nv{i}")
                for i in range(2 * RBANK)
            ]

            def _load_bank(blk):
                lo = blk * RBANK
                hi = min(nchunks, lo + RBANK)
                if lo >= hi:
                    return
                bank = [nv_regs[(blk % 2) * RBANK + j] for j in range(hi - lo)]
                nc.gpsimd.reg_load(bank, nv_sb[0:1, lo:hi])

            _load_bank(0)
            _load_bank(1)

            def _nv_reg(ci):
                return nv_regs[((ci // RBANK) % 2) * RBANK + ci % RBANK]

            def piece_of(t):
                for i in range(len(bndl) - 1):
                    if bndl[i] <= t < bndl[i + 1]:
                        return i, bndl[i]
                raise AssertionError(t)

            def _idxs_ap_of(t0c, nt):
                pi, pb = piece_of(t0c)
                assert t0c + nt <= bndl[pi + 1], "chunk straddles piece"
                return idx_p[pi][:, 8 * (t0c - pb) : 8 * (t0c - pb + nt)]

            gpools = [gpool_lo, gpool_hi]
            # One-time zero of every gather buffer: -1-trimmed (skipped) slots
            # are never DMA-written, and the segsum matmul contracts over them
            # (their one-hot column is zero) -- stale SBUF could be Inf/NaN and
            # 0*NaN = NaN, so first-touch must be a finite write.
            for hh in (0, 1):
                for i in range(6):
                    tz = gpools[hh].tile(
                        [P, cap[hh] * P], GDT, tag=f"xg{hh}", name=f"xgz{hh}_{i}"
                    )
                    nc.vector.memset(tz[:], 0.0)
            qn = 0
            for b in range(nb):
                groups = list(range(b * BUNDLE, min(ng, (b + 1) * BUNDLE)))
                xgc = [None, None]
                ct0 = [0, 0]
                for hh in (0, 1):
                    if (b, hh) not in chunk_of:
                        continue
                    t0c, nt, ci = chunk_of[(b, hh)]
                    ct0[hh] = t0c
                    xgc[hh] = gpools[hh].tile(
                        [P, cap[hh] * P], GDT, tag=f"xg{hh}", name=f"xg{hh}_{b}"
                    )
                    nc.gpsimd.dma_gather(
                        out_ap=xgc[hh][:, : nt * P].rearrange(
                            "p (k f) -> p k f", f=P
                        ),
                        in_ap=x_in[hh][:],
                        idxs_ap=_idxs_ap_of(t0c, nt),
                        num_idxs=nt * P,
                        num_idxs_reg=_nv_reg(ci),
                        elem_size=P,
                        queue_num=qn,
                        single_packet=False,
                    )
                    qn = (qn + 1) % 4
                    if ci % RBANK == RBANK - 1:
                        _load_bank(ci // RBANK + 2)

                ypsums = {}
                n_sub = {gg: int(nk2[gg][0] + nk2[gg][1]) for gg in groups}
                done = {gg: 0 for gg in groups}
                for gg in groups:
                    ypsums[gg] = yppool.tile(
                        [P, P], F32, tag="ypsum", name=f"ypsum_{gg}"
                    )
                for hh in (0, 1):
                    for gg in groups:
                        for k in range(int(nk2[gg][hh])):
                            t = int(seg_t0[gg][hh]) + k
                            koff = t - ct0[hh]
                            ind = ipool.tile([P, P], GDT, tag="ind",
                                             name=f"ind_{t}")
                            pi, pb = piece_of(t)
                            nc.vector.tensor_tensor(
                                out=ind[:],
                                in0=dst_p[pi][:, t - pb : t - pb + 1]
                                .to_broadcast([P, P]),
                                in1=iota_sb[:],
                                op=mybir.AluOpType.is_equal,
                            )
                            nc.tensor.matmul(
                                out=ypsums[gg][:],
                                lhsT=xgc[hh][:, koff * P : (koff + 1) * P],
                                rhs=ind[:],
                                start=(done[gg] == 0),
                                stop=(done[gg] == n_sub[gg] - 1),
                            )
                            done[gg] += 1

                for gg in groups:
                    nv = min(P, npc - gg * P)
                    ysb = ypool.tile([P, P], F32, tag="ysb", name=f"ysb_{gg}")
                    nc.scalar.copy(out=ysb[:], in_=ypsums[gg][:])
                    opsum = oppool.tile([P, P], F32, tag="opsum",
                                        name=f"opsum_{gg}")
                    nc.tensor.matmul(
                        out=opsum[:], lhsT=ysb[:], rhs=w4_sb[:],
                        start=True, stop=False,
                    )
                    nc.tensor.matmul(
                        out=opsum[:],
                        lhsT=deg4_sb[:, gg * P : (gg + 1) * P],
                        rhs=b4_sb[:],
                        start=False,
                        stop=True,
                    )
                    osb = opool.tile([P, P], F32, tag="osb", name=f"osb_{gg}")
                    nc.scalar.copy(out=osb[:], in_=opsum[:])
                    nc.sync.dma_start(
                        out=out_d[gg * P : gg * P + nv, :], in_=osb[:nv, :]
                    )

    nc.compile()
    return nc


def _run(inputs, trace=False, trace_kwargs=None):
    x = np.asarray(inputs["x"], np.float32)
    weight = np.asarray(inputs["weight"], np.float32)
    bias = np.asarray(inputs["bias"], np.float32)
    src = np.asarray(inputs["src"])
    dst = np.asarray(inputs["dst"])

    in_maps, meta = _prep(x, weight, bias, src, dst, N_CORES, NPC)
    nc = _build(N_NODES, D, NPC, meta)
    res = run_bass_kernel_spmd(
        nc,
        in_maps,
        list(range(N_CORES)),
        trace=trace,
        **(trace_kwargs or {}),
    )
    out = np.concatenate([res.results[c]["out"] for c in range(N_CORES)], axis=0)
    return out.reshape(N_NODES, HEADS, D // HEADS), res


def kernel(**inputs) -> np.ndarray:
    # the device occasionally comes up wedged from a prior run
    # (NRT_EXEC_UNIT_UNRECOVERABLE); a retry recovers it
    last = None
    for _ in range(3):
        try:
            out, _ = _run(inputs)
            return out
        except Exception as e:  # noqa: BLE001
            last = e
    raise last



# revision 22
# speedup vs baseline: 1.1178x; 1.1178x over previous
"""GAT layer kernel for Trainium2 (8 NeuronCores, SPMD).

Math note: in the reference,
    att = softmax(scores, axis=1); w = att.sum(axis=1)
sums the softmax over the exact axis it normalizes, so w == 1 identically
(up to fp rounding).  The layer therefore reduces to
    out[v] = (1/H) * ( (sum_{e: dst[e]==v} x[src[e]]) @ W  +  deg_in(v) * b )
i.e. a sparse neighbor aggregation (gather + segment-sum) followed by a
small dense matmul.  This is memory-bound in the per-edge row gather,
which is what the kernel optimizes.

Sharding: edges are partitioned by dst-node range (6250 nodes per core), so
each core owns the full accumulation for its node slice and the final
output is a pure concatenation -- no inter-core collective needed.

Device pipeline per core:
  - edges are grouped by 128-node output tile ("group") and src half
    (gather indices are int16, so x is split in two row halves); each
    (group, half) segment is padded to a multiple of 128 edge slots (pad
    slots gather row 0 and carry dst offset -1 so they contribute exactly
    nothing to the MACs).
  - the x table is fp16: halves HBM bytes; accumulation stays fp32, so
    the only loss is the one-off fp16 quantization of x (~2e-4 relative).
  - segments of BUNDLE adjacent groups share one dma_gather per half
    (~2K rows per call, the SWDGE sweet spot) spread over 4 SWDGE queues.
    The Q7 descriptor-prep rate (~2.6-3.3 ns/row on the two full-reach
    Q7 cores) is the hard floor of this kernel.
  - segment-sum on the tensor engine, one matmul per 128-edge subtile:
      ypsum_g[f, v] += xg[e, f]^T @ onehot[e, v]
    with the one-hot from a vector-engine is_equal of an iota row against
    the per-edge group-local dst offset.
  - per group, PSUM y^T is multiplied by W/4 (K=128 matmul) plus a K=1
    outer-product matmul for the deg*b/4 term, and streamed out.
"""

import numpy as np

import concourse.bass as bass
import concourse.tile as tile
from concourse import bacc, mybir
from concourse.bass_utils import run_bass_kernel_spmd

F32 = mybir.dt.float32
I16 = mybir.dt.int16
GDT = mybir.dt.float16   # gather-table / one-hot dtype
GNP = np.float16

N_NODES = 50000
N_EDGES = 800000
D = 128          # in feats == H*F
HEADS = 4
N_CORES = 8
NPC = N_NODES // N_CORES      # nodes per core
P = 128                       # partitions / node-group size
BUNDLE = 2                    # groups whose segments share one gather


def _prep(x, weight, bias, src, dst, n_cores, npc):
    """Host-side sharding; returns per-core input maps + static tiling."""
    n_nodes, d = x.shape
    ng = (npc + P - 1) // P
    half = (n_nodes + 1) // 2
    assert half < 32768 and (n_nodes - half) < 32768

    src64 = src.astype(np.int64)
    dst64 = dst.astype(np.int64)
    core = dst64 // npc
    loc_node = dst64 % npc
    g_e = loc_node // P
    loc = (loc_node % P).astype(np.float32)
    h_e = (src64 >= half).astype(np.int64)

    key = (core * ng + g_e) * 2 + h_e
    order = np.argsort(key, kind="stable")
    key_s = key[order]
    src_s = src64[order]
    loc_s = loc[order]

    cnt = np.bincount(key, minlength=n_cores * ng * 2).reshape(n_cores, ng, 2)
    nk2 = (-(-cnt.max(axis=0) // P)).astype(np.int64)  # [ng, 2] tiles/segment

    # Bundle composition: pairs of groups share a gather (amortizes per-call
    # cost); the LAST few groups get single-group bundles so the end-of-run
    # drain chain (gather -> matmuls -> W -> out) is short.
    nsingle = 2 + (ng % 2)
    bundles = [[2 * i, 2 * i + 1] for i in range((ng - nsingle) // 2)]
    bundles += [[g] for g in range(ng - nsingle, ng)]
    nb = len(bundles)

    # subtile order: per bundle, lo segments then hi
    seg_t0 = np.zeros((ng, 2), np.int64)
    chunks = []  # (bundle, half, t0, ntiles)
    t = 0
    for b, gb in enumerate(bundles):
        for hh in (0, 1):
            t0c = t
            for gg in gb:
                seg_t0[gg, hh] = t
                t += int(nk2[gg, hh])
            if t > t0c:
                chunks.append((b, hh, t0c, t - t0c))
    T = t

    seg_start = np.zeros(n_cores * ng * 2, np.int64)
    seg_start[1:] = np.cumsum(cnt.reshape(-1))[:-1]
    rank = np.arange(len(src_s), dtype=np.int64) - seg_start[key_s]
    c_e = key_s // (ng * 2)
    gg_e = (key_s // 2) % ng
    t_e = seg_t0[gg_e, key_s % 2] + rank // P
    p_e = rank % P

    dstoff = np.full((n_cores, P, T), -1.0, GNP)
    dstoff[c_e, p_e, t_e] = loc_s.astype(GNP)

    # int16 indices: slot (p, t) -> idx16[p % 16, 8*t + p//16]
    # (pad slots keep idx 0: they gather row 0 and their one-hot column is
    # zero.  Skipping them via trailing -1 idxs was tried and is a net LOSS:
    # the ucode trims trailing negatives with a SCALAR loop at ~3ns/row,
    # more than the ~2.6ns/row the vectorized desc-gen saves.)
    idx16 = np.zeros((n_cores, 16, 8 * T), np.int16)
    hval = (src_s - (key_s % 2) * half).astype(np.int16)
    idx16[c_e, p_e % 16, 8 * t_e + p_e // 16] = hval
    idx16 = np.ascontiguousarray(np.tile(idx16, (1, 8, 1)))

    deg = np.bincount(dst64, minlength=n_nodes).astype(np.float32)
    deg4 = np.zeros((n_cores, 1, ng * P), np.float32)
    deg4[:, 0, :npc] = (deg / HEADS).reshape(n_cores, npc)

    iota = np.broadcast_to(np.arange(P, dtype=GNP)[None, :], (P, P)).copy()
    w4 = np.ascontiguousarray(weight.astype(np.float32) / HEADS)
    b4 = bias.astype(np.float32).reshape(1, d)  # deg4 already carries the /H
    xg16 = np.ascontiguousarray(x.astype(GNP))
    xlo = np.ascontiguousarray(xg16[:half])
    xhi = np.ascontiguousarray(xg16[half:])

    in_maps = []
    for c in range(n_cores):
        in_maps.append(
            {
                "xlo": xlo,
                "xhi": xhi,
                "idx": idx16[c],
                "dstoff": dstoff[c],
                "iota": iota,
                "w4": w4,
                "b4": b4,
                "deg4": deg4[c],
            }
        )
    meta = dict(nk2=nk2, seg_t0=seg_t0, chunks=chunks, T=T, ng=ng,
                bundles=bundles, half=half)
    return in_maps, meta


def _build(n_nodes, d, npc, meta):
    nk2, seg_t0, chunks, T, ng, bundles, half = (
        meta["nk2"], meta["seg_t0"], meta["chunks"], meta["T"],
        meta["ng"], meta["bundles"], meta["half"],
    )
    nc = bacc.Bacc("TRN2", num_swdge_queues=4)
    xlo_d = nc.dram_tensor("xlo", [half, d], GDT, kind="ExternalInput")
    xhi_d = nc.dram_tensor("xhi", [n_nodes - half, d], GDT, kind="ExternalInput")
    idx_d = nc.dram_tensor("idx", [P, 8 * T], I16, kind="ExternalInput")
    dstoff_d = nc.dram_tensor("dstoff", [P, T], GDT, kind="ExternalInput")
    iota_d = nc.dram_tensor("iota", [P, P], GDT, kind="ExternalInput")
    w4_d = nc.dram_tensor("w4", [d, d], F32, kind="ExternalInput")
    b4_d = nc.dram_tensor("b4", [1, d], F32, kind="ExternalInput")
    deg4_d = nc.dram_tensor("deg4", [1, ng * P], F32, kind="ExternalInput")
    out_d = nc.dram_tensor("out", [npc, d], F32, kind="ExternalOutput")

    x_in = [xlo_d, xhi_d]
    cap = [1, 1]
    for (_b, hh, _t0, nt) in chunks:
        cap[hh] = max(cap[hh], nt)
    chunk_of = {}
    for (b, hh, t0c, nt) in chunks:
        chunk_of[(b, hh)] = (t0c, nt)

    with tile.TileContext(nc) as tc:
        with (
            tc.tile_pool(name="consts", bufs=1) as cpool,
            tc.tile_pool(name="xglo", bufs=6) as gpool_lo,
            tc.tile_pool(name="xghi", bufs=6) as gpool_hi,
            tc.tile_pool(name="ind", bufs=32) as ipool,
            tc.tile_pool(name="ysb", bufs=4) as ypool,
            tc.tile_pool(name="osb", bufs=4) as opool,
            tc.tile_pool(name="ypsum", bufs=6, space="PSUM") as yppool,
            tc.tile_pool(name="opsum", bufs=2, space="PSUM") as oppool,
        ):
            # piecewise index tables as INDEPENDENT tiles (Tile tracks deps
            # per tile, so the first gather must not wait on the whole
            # table).  Piece boundaries align with chunk boundaries and are
            # GEOMETRIC: tiny first pieces so gather #0 starts ~2us in, big
            # later pieces once there's pipeline runway.
            target = 4
            bndl = [0]
            for (_b, _hh, t0c, nt) in chunks:
                if t0c + nt - bndl[-1] >= target and t0c + nt < T:
                    bndl.append(t0c + nt)
                    target = min(target * 2, 160)
            bndl.append(T)
            idx_p, dst_p = [], []
            for i in range(len(bndl) - 1):
                lo, hi = bndl[i], bndl[i + 1]
                ip = cpool.tile([P, 8 * (hi - lo)], I16, name=f"idxp{i}")
                dp = cpool.tile([P, hi - lo], GDT, name=f"dstp{i}")
                idx_p.append(ip)
                dst_p.append(dp)

            iota_sb = cpool.tile([P, P], GDT)
            w4_sb = cpool.tile([d, d], F32)
            b4_sb = cpool.tile([1, d], F32)
            deg4_sb = cpool.tile([1, ng * P], F32)

            # DMA issue order/queues: the first idx piece rides the sync
            # queue alone (gates gather #0); dstoff/iota (gate the one-hot
            # builds) go on the vector queue, other constants on scalar --
            # three sequencers dispatch in parallel at program start.
            nc.sync.dma_start(out=idx_p[0][:], in_=idx_d[:, : 8 * bndl[1]])
            nc.scalar.dma_start(out=dst_p[0][:], in_=dstoff_d[:, : bndl[1]])
            nc.scalar.dma_start(out=iota_sb[:], in_=iota_d[:])
            nc.scalar.dma_start(out=w4_sb[:], in_=w4_d[:])
            nc.scalar.dma_start(out=b4_sb[:], in_=b4_d[:])
            nc.scalar.dma_start(out=deg4_sb[:], in_=deg4_d[:])
            for i in range(1, len(idx_p)):
                lo, hi = bndl[i], bndl[i + 1]
                nc.sync.dma_start(out=idx_p[i][:], in_=idx_d[:, 8 * lo : 8 * hi])
                nc.scalar.dma_start(out=dst_p[i][:], in_=dstoff_d[:, lo:hi])

            def piece_of(t):
                for i in range(len(bndl) - 1):
                    if bndl[i] <= t < bndl[i + 1]:
                        return i, bndl[i]
                raise AssertionError(t)

            def _idxs_ap_of(t0c, nt):
                pi, pb = piece_of(t0c)
                assert t0c + nt <= bndl[pi + 1], "chunk straddles piece"
                return idx_p[pi][:, 8 * (t0c - pb) : 8 * (t0c - pb + nt)]

            gpools = [gpool_lo, gpool_hi]
            qn = 0
            for b, groups in enumerate(bundles):
                xgc = [None, None]
                ct0 = [0, 0]
                for hh in (0, 1):
                    if (b, hh) not in chunk_of:
                        continue
                    t0c, nt = chunk_of[(b, hh)]
                    ct0[hh] = t0c
                    xgc[hh] = gpools[hh].tile(
                        [P, cap[hh] * P], GDT, tag=f"xg{hh}", name=f"xg{hh}_{b}"
                    )
                    nc.gpsimd.dma_gather(
                        out_ap=xgc[hh][:, : nt * P].rearrange(
                            "p (k f) -> p k f", f=P
                        ),
                        in_ap=x_in[hh][:],
                        idxs_ap=_idxs_ap_of(t0c, nt),
                        num_idxs=nt * P,
                        num_idxs_reg=nt * P,
                        elem_size=P,
                        queue_num=qn,
                        single_packet=False,
                    )
                    qn = (qn + 1) % 4

                ypsums = {}
                n_sub = {gg: int(nk2[gg][0] + nk2[gg][1]) for gg in groups}
                done = {gg: 0 for gg in groups}
                for gg in groups:
                    ypsums[gg] = yppool.tile(
                        [P, P], F32, tag="ypsum", name=f"ypsum_{gg}"
                    )
                for hh in (0, 1):
                    for gg in groups:
                        for k in range(int(nk2[gg][hh])):
                            t = int(seg_t0[gg][hh]) + k
                            koff = t - ct0[hh]
                            ind = ipool.tile([P, P], GDT, tag="ind",
                                             name=f"ind_{t}")
                            pi, pb = piece_of(t)
                            nc.vector.tensor_tensor(
                                out=ind[:],
                                in0=dst_p[pi][:, t - pb : t - pb + 1]
                                .to_broadcast([P, P]),
                                in1=iota_sb[:],
                                op=mybir.AluOpType.is_equal,
                            )
                            nc.tensor.matmul(
                                out=ypsums[gg][:],
                                lhsT=xgc[hh][:, koff * P : (koff + 1) * P],
                                rhs=ind[:],
                                start=(done[gg] == 0),
                                stop=(done[gg] == n_sub[gg] - 1),
                            )
                            done[gg] += 1

                for gg in groups:
                    nv = min(P, npc - gg * P)
                    ysb = ypool.tile([P, P], F32, tag="ysb", name=f"ysb_{gg}")
                    nc.scalar.copy(out=ysb[:], in_=ypsums[gg][:])
                    opsum = oppool.tile([P, P], F32, tag="opsum",
                                        name=f"opsum_{gg}")
                    nc.tensor.matmul(
                        out=opsum[:], lhsT=ysb[:], rhs=w4_sb[:],
                        start=True, stop=False,
                    )
                    nc.tensor.matmul(
                        out=opsum[:],
                        lhsT=deg4_sb[:, gg * P : (gg + 1) * P],
                        rhs=b4_sb[:],
                        start=False,
                        stop=True,
                    )
                    osb = opool.tile([P, P], F32, tag="osb", name=f"osb_{gg}")
                    nc.scalar.copy(out=osb[:], in_=opsum[:])
                    nc.sync.dma_start(
                        out=out_d[gg * P : gg * P + nv, :], in_=osb[:nv, :]
                    )

    nc.compile()
    return nc


def _run(inputs, trace=False, trace_kwargs=None):
    x = np.asarray(inputs["x"], np.float32)
    weight = np.asarray(inputs["weight"], np.float32)
    bias = np.asarray(inputs["bias"], np.float32)
    src = np.asarray(inputs["src"])
    dst = np.asarray(inputs["dst"])

    in_maps, meta = _prep(x, weight, bias, src, dst, N_CORES, NPC)
    nc = _build(N_NODES, D, NPC, meta)
    res = run_bass_kernel_spmd(
        nc,
        in_maps,
        list(range(N_CORES)),
        trace=trace,
        **(trace_kwargs or {}),
    )
    out = np.concatenate([res.results[c]["out"] for c in range(N_CORES)], axis=0)
    return out.reshape(N_NODES, HEADS, D // HEADS), res


def kernel(**inputs) -> np.ndarray:
    # the device occasionally comes up wedged from a prior run
    # (NRT_EXEC_UNIT_UNRECOVERABLE); a retry recovers it
    last = None
    for _ in range(3):
        try:
            out, _ = _run(inputs)
            return out
        except Exception as e:  # noqa: BLE001
            last = e
    raise last

